# revision 35
# baseline (speedup 1.0000x reference)
"""Trainium2 Bass kernel for nn_BGguidedLoss (BG-guided loss function).

Strategy: pure data-parallel over 8 NeuronCores. Each core processes
N/8 = 524288 rays laid out as [128 partitions x 4096 rays/partition].

Per-ray math (matching the jax reference semantics exactly):
  - HSV hue/value of gt and BG_map via a branchless select-free form of
    the skimage piecewise hue (Hocevar-style), including this jax
    build's `%` semantics (x % 1.0 == x - round_half_away(x)) and the
    exact tie priority of the reference where-chain.
  - diff = sqrt(dh^2 + dv^2), mask = sigmoid((diff - threshold)/0.1)
  - BG/FG MSE terms, FG uncertainty scaling, masked means.

Work is spread across engines: hue chain on DVE, squares and all
transcendentals (incl. 1/(6d) = exp(-ln(6d))) on ScalarE/ACT, the MSE
difference chain and final combine on GPSIMD.  Each core returns 128
per-partition partial sums; the host sums them and divides by N.
threshold_param enters via a host-computed per-partition bias constant.
"""

import numpy as np

N_TOTAL = 4194304
N_CORES = 8
NC_RAYS = N_TOTAL // N_CORES          # 524288 rays per core
P = 128                               # partitions
FPP = NC_RAYS // P                    # 4096 rays per partition
K = 512                               # rays per partition per tile
NIT = FPP // K                        # tile iterations
EPS = float(2.0 ** -33)
BIAS_TINY = float(2.0 ** -30)
LN6INV = float(np.log(np.float32(1.0 / 6.0)))
USE_GPSIMD = True
GP_CROSS = True
GP_MD = False

_CACHE = {}


def _build(full_variant: bool):
    import concourse.bacc as bacc
    import concourse.mybir as mybir
    import concourse.tile as tile

    f32 = mybir.dt.float32
    op = mybir.AluOpType
    act = mybir.ActivationFunctionType

    nc = bacc.Bacc("TRN2", debug=False)

    # register a tiny-constant AP so activation() accepts it as a bias
    _ct = nc.alloc_sbuf_tensor("const-f32-tiny", [128, 1], f32)
    nc.gpsimd.memset(_ct.ap(), BIAS_TINY)
    nc.const_aps.aps[(f32, BIAS_TINY)] = _ct.ap()

    gt_d = nc.dram_tensor("gt_s", [NC_RAYS, 3], f32, kind="ExternalInput")
    bg_d = nc.dram_tensor("bg_s", [NC_RAYS, 3], f32, kind="ExternalInput")
    out_d = nc.dram_tensor("out_s", [P], f32, kind="ExternalOutput")
    if full_variant:
        fg_d = nc.dram_tensor("fg_s", [NC_RAYS, 3], f32, kind="ExternalInput")
        u_d = nc.dram_tensor("u_s", [NC_RAYS], f32, kind="ExternalInput")
        prm_d = nc.dram_tensor("prm_s", [P, 2], f32, kind="ExternalInput")

    gt_v = gt_d.ap().rearrange("(p f) c -> p (f c)", p=P)
    bg_v = bg_d.ap().rearrange("(p f) c -> p (f c)", p=P)
    if full_variant:
        fg_v = fg_d.ap().rearrange("(p f) c -> p (f c)", p=P)
        u_v = u_d.ap().rearrange("(p f) -> p f", p=P)
    out_v = out_d.ap().rearrange("(p o) -> p o", o=1)

    TT = None

    with tile.TileContext(nc) as tc:
        with (
            tc.tile_pool(name="pin", bufs=2) as pin,
            tc.tile_pool(name="ptmp", bufs=1) as ptmp,
            tc.tile_pool(name="pers", bufs=1) as pers,
        ):
            TT = nc.vector.tensor_tensor
            STT = nc.vector.scalar_tensor_tensor
            GTT = nc.gpsimd.tensor_tensor if USE_GPSIMD else TT
            if not full_variant:
                accT = pers.tile([P, 1], f32, tag="accT")
                nc.vector.memset(accT, 0.0)
                for t in range(NIT):
                    sl = slice(t * 3 * K, (t + 1) * 3 * K)
                    g = pin.tile([P, 3 * K], f32, tag="g", name=f"g{t}")
                    b = pin.tile([P, 3 * K], f32, tag="b", name=f"b{t}")
                    nc.sync.dma_start(g, gt_v[:, sl])
                    nc.sync.dma_start(b, bg_v[:, sl])
                    e = ptmp.tile([P, 3 * K], f32, tag="e", bufs=2,
                                  name=f"e{t}")
                    TT(e, g, b, op.subtract)
                    nc.scalar.activation(e, e, act.Square)
                    acc_t = ptmp.tile([P, 1], f32, tag="acc_t", bufs=2,
                                      name=f"acc{t}")
                    nc.vector.tensor_scalar(e, e, 1.0, None, op.mult,
                                            op.add, accum_out=acc_t)
                    TT(accT, accT, acc_t, op.add)
                nc.sync.dma_start(out_v, accT)
            else:
                sArr = pers.tile([P, FPP], f32, tag="sArr")
                bArr = pers.tile([P, FPP], f32, tag="bArr")
                fArr = pers.tile([P, FPP], f32, tag="fArr")
                uArr = pers.tile([P, FPP], f32, tag="uArr")
                eArr = pers.tile([P, FPP], f32, tag="eArr")   # scratch
                prm = pers.tile([P, 2], f32, tag="prm")
                nc.sync.dma_start(prm, prm_d.ap())
                nc.sync.dma_start(uArr, u_v)

                for t in range(NIT):
                    sl3 = slice(t * 3 * K, (t + 1) * 3 * K)
                    sl1 = slice(t * K, (t + 1) * K)
                    gb = pin.tile([P, 6 * K], f32, tag="gb", name=f"gb{t}")
                    ff = pin.tile([P, 3 * K], f32, tag="ff", name=f"ff{t}")
                    nc.sync.dma_start(gb[:, :3 * K], gt_v[:, sl3])
                    nc.sync.dma_start(gb[:, 3 * K:], bg_v[:, sl3])
                    nc.sync.dma_start(ff, fg_v[:, sl3])

                    gbv = gb.rearrange("p (i k c) -> p i k c", i=2, c=3)
                    r = gbv[:, :, :, 0]
                    g = gbv[:, :, :, 1]
                    b = gbv[:, :, :, 2]

                    def t2k(nm, tag="h2k", bufs=9):
                        return ptmp.tile([P, 2 * K], f32, tag=tag,
                                         bufs=bufs, name=f"{nm}{t}"
                                         ).rearrange("p (i k) -> p i k", i=2)

                    # hue chain (batched over gt|BG in [p,2,K] views);
                    # heavy in-place reuse to fit SBUF
                    Px = t2k("Px"); TT(Px, g, b, op.max)
                    c1 = t2k("c1"); TT(c1, g, b, op.is_lt)
                    Py = t2k("Py"); TT(Py, g, b, op.min)
                    c2 = t2k("c2"); TT(c2, r, Px, op.is_lt)
                    x = t2k("x"); TT(x, c1, c2, op.not_equal)
                    zc = t2k("zc")
                    STT(zc, c1, 1.0, c2, op.add, op.mult)
                    M = t2k("M", tag="Mt", bufs=2)
                    TT(M, r, Px, op.max)
                    Qw = t2k("Qw"); TT(Qw, r, Px, op.min)
                    MD = GTT if GP_MD else TT
                    m = t2k("m"); MD(m, Qw, Py, op.min)
                    MD(m, M, m, op.subtract)            # m <- d
                    # rc = 1/(6d + tiny) = exp(-ln(6d + tiny)); tiny keeps
                    # d == 0 finite (num == 0 there, so nq stays 0)
                    sd = t2k("sd")
                    sdf = sd.rearrange("p i k -> p (i k)")
                    nc.scalar.activation(sdf,
                                         m.rearrange("p i k -> p (i k)"),
                                         act.Ln, scale=6.0, bias=BIAS_TINY)
                    nc.scalar.activation(sdf, sdf, act.Exp, scale=-1.0)
                    num = t2k("num"); TT(num, Qw, Py, op.subtract)
                    TT(num, num, sd, op.mult)           # num <- nq
                    t1 = t2k("t1"); TT(t1, x, num, op.mult)
                    STT(t1, t1, -2.0, num, op.mult, op.add)   # t1 <- q2
                    STT(zc, zc, 1.0 / 3.0, t1, op.mult, op.add)  # zc <- hp
                    nh = t2k("nh", tag="nht", bufs=2)
                    # nh = [hp >= 0.5] - hp   (= -h_ref)
                    STT(nh, zc, 0.5, zc, op.is_ge, op.subtract)

                    # cross terms -> sArr  (DVE + ACT squares)
                    dh = ptmp.tile([P, K], f32, tag="dh", bufs=2,
                                   name=f"dh{t}")
                    dv = ptmp.tile([P, K], f32, tag="dv", bufs=2,
                                   name=f"dv{t}")
                    CR = GTT if GP_CROSS else TT
                    CR(dh, nh[:, 1, :], nh[:, 0, :], op.subtract)
                    CR(dv, M[:, 0, :], M[:, 1, :], op.subtract)
                    nc.scalar.activation(dh, dh, act.Square)
                    nc.scalar.activation(dv, dv, act.Square)
                    CR(sArr[:, sl1], dh, dv, op.add)

                    # MSE terms -> bArr, fArr  (GPSIMD + ACT squares)
                    for (dst, other) in ((bArr, gb[:, 3 * K:]), (fArr, ff)):
                        e = ptmp.tile([P, 3 * K], f32, tag="e3k", bufs=2,
                                      name=f"e{t}")
                        GTT(e, gb[:, :3 * K], other, op.subtract)
                        # square + channel-deinterleave in one ACT pass:
                        # esq[p, c*K+k] = e[p, 3k+c]^2 (strided ACT read is
                        # free; gives GPSIMD contiguous adds below)
                        esq = ptmp.tile([P, 3 * K], f32, tag="esq", bufs=2,
                                        name=f"esq{t}")
                        ev = esq.rearrange("p (c k) -> p c k", c=3)
                        nc.scalar.activation(
                            ev, e.rearrange("p (k c) -> p c k", c=3),
                            act.Square)
                        q01 = ptmp.tile([P, K], f32, tag="q01", bufs=2,
                                        name=f"q01{t}")
                        GTT(q01, ev[:, 0, :], ev[:, 1, :], op.add)
                        GTT(dst[:, sl1], q01, ev[:, 2, :], op.add)

                # ---- phase 2: batched transcendentals
                nc.scalar.activation(sArr, sArr, act.Sqrt)
                nc.scalar.activation(sArr, sArr, act.Sigmoid,
                                     bias=prm[:, 0:1], scale=10.0)
                nc.scalar.activation(uArr, uArr, act.Ln)
                # eArr = exp(-2 ln u + ln(1/6)) = 1/(6 u^2)
                nc.scalar.activation(eArr, uArr, act.Exp,
                                     bias=prm[:, 1:2], scale=-2.0)

                # ---- phase 3: combine + reduce
                GTT(fArr, fArr, eArr, op.mult)      # ssqF/(6u^2)
                STT(fArr, bArr, -1.0 / 3.0, fArr, op.mult, op.add)
                GTT(fArr, fArr, uArr, op.add)       # C
                GTT(fArr, fArr, sArr, op.mult)      # C * mask
                accP = pers.tile([P, 1], f32, tag="accP")
                accB = pers.tile([P, 1], f32, tag="accB")
                nc.vector.tensor_scalar(eArr, fArr, 1.0, None,
                                        op.mult, op.add, accum_out=accP)
                nc.vector.tensor_scalar(eArr, bArr, 1.0 / 3.0, None,
                                        op.mult, op.add, accum_out=accB)
                TT(accP, accP, accB, op.add)
                nc.sync.dma_start(out_v, accP)

    nc.compile()
    return nc


def _get_nc(full_variant: bool):
    key = bool(full_variant)
    if key not in _CACHE:
        _CACHE[key] = _build(full_variant)
    return _CACHE[key]


def _run(inputs, trace=False):
    from concourse.bass_utils import run_bass_kernel_spmd

    gt = np.ascontiguousarray(np.asarray(inputs["gt"], dtype=np.float32))
    bg = np.ascontiguousarray(np.asarray(inputs["BG_map"], dtype=np.float32))
    it = int(np.asarray(inputs["iter"]))
    full = it > 300

    if full:
        fg = np.ascontiguousarray(np.asarray(inputs["FG_map"],
                                             dtype=np.float32))
        u = np.ascontiguousarray(
            np.asarray(inputs["FG_uncertainties"], dtype=np.float32)
        ).reshape(-1)
        tp = float(np.asarray(inputs["threshold_param"]))
        thr = 1.414 * (1.0 - 1.0 / (1.0 + np.exp(-tp)))
        prm = np.zeros((P, 2), dtype=np.float32)
        prm[:, 0] = np.float32(-10.0 * thr)
        prm[:, 1] = np.float32(LN6INV)

    nc = _get_nc(full)
    in_maps = []
    for c in range(N_CORES):
        sl = slice(c * NC_RAYS, (c + 1) * NC_RAYS)
        m = {"gt_s": gt[sl], "bg_s": bg[sl]}
        if full:
            m["fg_s"] = fg[sl]
            m["u_s"] = u[sl]
            m["prm_s"] = prm
        in_maps.append(m)

    res = run_bass_kernel_spmd(nc, in_maps, core_ids=list(range(N_CORES)),
                               trace=trace)
    parts = np.stack([r["out_s"] for r in res.results])  # [8, 128]
    total = parts.astype(np.float64).sum()
    if full:
        val = total / N_TOTAL
    else:
        val = total / (N_TOTAL * 3)
    return np.float32(val), res


def kernel(**inputs) -> np.ndarray:
    val, _ = _run(inputs, trace=False)
    return np.asarray(val, dtype=np.float32)


# ---------------------------------------------------------------------------
# Timing helper (test harness only): cached sharded executable + resident
# inputs; min wall over repeats approximates per-launch HW time + RPC.
def _hw_time(inputs, iters=10):
    import time
    import jax
    import numpy as _np
    from jax.sharding import Mesh, PartitionSpec, NamedSharding
    from jax.experimental.shard_map import shard_map
    import concourse.mybir as mybir
    from concourse import bass2jax

    gt = np.asarray(inputs["gt"], dtype=np.float32)
    bg = np.asarray(inputs["BG_map"], dtype=np.float32)
    fg = np.asarray(inputs["FG_map"], dtype=np.float32)
    u = np.asarray(inputs["FG_uncertainties"], dtype=np.float32).reshape(-1)
    tp = float(np.asarray(inputs["threshold_param"]))
    thr = 1.414 * (1.0 - 1.0 / (1.0 + np.exp(-tp)))
    prm = np.zeros((P, 2), dtype=np.float32)
    prm[:, 0] = np.float32(-10.0 * thr)
    prm[:, 1] = np.float32(LN6INV)
    prm_all = np.tile(prm, (N_CORES, 1))

    nc = _get_nc(True)
    bass2jax.install_neuronx_cc_hook()

    part_name = (nc.partition_id_tensor.name
                 if nc.partition_id_tensor else None)
    in_names, out_names, out_avals = [], [], []
    for alloc in nc.m.functions[0].allocations:
        if not isinstance(alloc, mybir.MemoryLocationSet):
            continue
        name = alloc.memorylocations[0].name
        if alloc.kind == "ExternalInput":
            if name != part_name:
                in_names.append(name)
        elif alloc.kind == "ExternalOutput":
            out_names.append(name)
            out_avals.append(jax.core.ShapedArray(
                tuple(alloc.tensor_shape), mybir.dt.np(alloc.dtype)))
    n_params = len(in_names)
    in_names = in_names + out_names
    if part_name is not None:
        in_names.append(part_name)
    donate = tuple(range(n_params, n_params + len(out_names)))

    def _body(*args):
        operands = list(args)
        if part_name is not None:
            operands.append(bass2jax.partition_id_tensor())
        outs = bass2jax._bass_exec_p.bind(
            *operands, out_avals=tuple(out_avals), in_names=tuple(in_names),
            out_names=tuple(out_names), lowering_input_output_aliases=(),
            sim_require_finite=True, sim_require_nnan=True, nc=nc)
        return tuple(outs)

    devices = jax.devices()[:N_CORES]
    mesh = Mesh(_np.asarray(devices), ("core",))
    spec = PartitionSpec("core")
    n_out = len(out_names)
    sharded = jax.jit(
        shard_map(_body, mesh=mesh, in_specs=(spec,) * (n_params + n_out),
                  out_specs=(spec,) * n_out, check_rep=False),
        donate_argnums=donate, keep_unused=True)

    full_in = {"gt_s": gt, "bg_s": bg, "fg_s": fg, "u_s": u,
               "prm_s": prm_all}
    sh = NamedSharding(mesh, spec)
    dev_in = [jax.device_put(full_in[n], sh) for n in in_names[:n_params]]
    zeros = [np.zeros((N_CORES * a.shape[0], *a.shape[1:]), a.dtype)
             for a in out_avals]

    # warmup
    out = sharded(*dev_in, *[jax.device_put(z, sh) for z in zeros])
    jax.block_until_ready(out)
    best = float("inf")
    for _ in range(iters):
        zin = [jax.device_put(z, sh) for z in zeros]
        jax.block_until_ready(zin)
        t0 = time.perf_counter()
        out = sharded(*dev_in, *zin)
        jax.block_until_ready(out)
        dt = time.perf_counter() - t0
        best = min(best, dt)
    return best, out
